# revision 1
# baseline (speedup 1.0000x reference)
"""Trainium2 distributed attention kernel for nn_Attention_72095321030782.

B=16, S=1024, DIM=1024, H=16, HD=64. Batch data-parallel over 8 cores
(2 batches/core), no collectives. Per core:
  x[2048,1024] -> transpose -> qkv proj -> QK RMSNorm + 2D RoPE ->
  per-(b,h) transposed-scores attention (S^T = k^T.T @ q^T, exp, PV with
  ones-column appended to v for the softmax denominator) -> out proj.
All matmuls run as float32r (full-rate fp32 path on the PE).
"""

import math
from contextlib import ExitStack

import numpy as np

import concourse.bass as bass
import concourse.tile as tile
from concourse import bacc, mybir
from concourse.bass_utils import run_bass_kernel_spmd

B, S, DIM, H = 16, 1024, 1024, 16
HD = DIM // H            # 64
ROPE_DIM = HD // 2       # 32
FT, PT_LEN = 32, 16
THETA = 10000.0
EPS = 1e-6
NCORES = 8
BL = B // NCORES         # 2 batches per core
T = BL * S               # 2048 tokens per core
NT = T // 128            # 16 token tiles
ND = DIM // 128          # 8 dim chunks
F32 = mybir.dt.float32
F32R = mybir.dt.float32r


def _rope_tables():
    freqs = 1.0 / THETA ** (np.arange(0, ROPE_DIM, 2, dtype=np.float32) / ROPE_DIM)
    t = np.arange(FT, dtype=np.float32) / FT * PT_LEN
    fs = np.einsum('n,f->nf', t, freqs).astype(np.float32)
    fs = np.repeat(fs, 2, axis=-1)                       # [FT, 32]
    fh = np.broadcast_to(fs[:, None, :], (FT, FT, ROPE_DIM))
    fw = np.broadcast_to(fs[None, :, :], (FT, FT, ROPE_DIM))
    f = np.concatenate([fh, fw], axis=-1).reshape(S, HD)
    return np.cos(f).astype(np.float32), np.sin(f).astype(np.float32)


def build_graph():
    nc = bacc.Bacc('TRN2', target_bir_lowering=False, debug=False,
                   num_devices=NCORES)
    x_e = nc.declare_dram_parameter('x', [T, DIM], F32, isOutput=False)
    wqkv_e = nc.declare_dram_parameter('w_qkv', [DIM, 3 * DIM], F32R, isOutput=False)
    wout_e = nc.declare_dram_parameter('w_out', [DIM, DIM], F32R, isOutput=False)
    bqkv_e = nc.declare_dram_parameter('b_qkv_t', [128, 3 * DIM], F32, isOutput=False)
    bout_e = nc.declare_dram_parameter('b_out_t', [128, DIM], F32, isOutput=False)
    cos_e = nc.declare_dram_parameter('cos16', [S // 128, 128, H, HD], F32, isOutput=False)
    sin_e = nc.declare_dram_parameter('sin16', [S // 128, 128, H, HD], F32, isOutput=False)
    ident_e = nc.declare_dram_parameter('ident', [128, 128], F32, isOutput=False)
    qnw_e = nc.declare_dram_parameter('qn_w', [128, H, HD], F32, isOutput=False)
    knw_e = nc.declare_dram_parameter('kn_w', [128, H, HD], F32, isOutput=False)
    out_e = nc.declare_dram_parameter('out', [T, DIM], F32, isOutput=True)

    x_ap = x_e.ap()
    out_ap = out_e.ap()

    with nc.allow_low_precision(reason='f32r matmul operands are rounded by design'), \
         tile.TileContext(nc) as tc, ExitStack() as ctx:
        dram = ctx.enter_context(tc.tile_pool(name='dram', bufs=1, space='DRAM'))
        qkT_d = dram.tile([2 * DIM, T], F32R)      # q^T rows 0:1024, k^T rows 1024:2048
        v_d = dram.tile([T, DIM], F32R)
        attnT_d = dram.tile([DIM, T], F32R)

        const = ctx.enter_context(tc.tile_pool(name='const', bufs=1))
        ident = const.tile([128, 128], F32)
        nc.sync.dma_start(ident[:], ident_e.ap()[:])
        ones_f32 = const.tile([1, HD], F32)
        nc.vector.memset(ones_f32[:], 1.0)
        ones_row = const.tile([1, HD], F32R)
        nc.scalar.activation(ones_row[:], ones_f32[:],
                             mybir.ActivationFunctionType.Copy)
        ones_col = const.tile([128, 1], F32)
        nc.vector.memset(ones_col[:], 1.0)
        qn_w = const.tile([128, H, HD], F32)
        kn_w = const.tile([128, H, HD], F32)
        nc.sync.dma_start(qn_w[:], qnw_e.ap()[:])
        nc.sync.dma_start(kn_w[:], knw_e.ap()[:])

        # ---- Phase 1: QKV + norm + rope, streaming over 16 token tiles ----
        wpool = ctx.enter_context(tc.tile_pool(name='wqkv', bufs=1))
        w_sb = []           # w_qkv resident: 8 chunks [128, 3072] = 96KB/part
        for d in range(ND):
            wt = wpool.tile([128, 3 * DIM], F32R, tag=f'w{d}')
            nc.sync.dma_start(wt[:], wqkv_e.ap()[bass.ts(d, 128), :])
            w_sb.append(wt)

        with tc.tile_pool(name='xin', bufs=2) as xin, \
             tc.tile_pool(name='xtp', bufs=2, space='PSUM') as xtp, \
             tc.tile_pool(name='xts', bufs=1) as xts, \
             tc.tile_pool(name='bqs', bufs=2) as bqs, \
             tc.tile_pool(name='qkvp', bufs=3, space='PSUM') as qkvp, \
             tc.tile_pool(name='qkvs', bufs=2) as qkvs, \
             tc.tile_pool(name='nrm', bufs=2) as nrm, \
             tc.tile_pool(name='cs', bufs=1) as cspool, \
             tc.tile_pool(name='rot', bufs=2) as rot, \
             tc.tile_pool(name='htp', bufs=2, space='PSUM') as htp, \
             tc.tile_pool(name='hts', bufs=3) as hts:
            for it in range(NT):
                pos_tile = it % (S // 128)
                x_t = xin.tile([128, DIM], F32, tag='x')
                nc.sync.dma_start(x_t[:], x_ap[bass.ts(it, 128), :])
                # transpose x tile: 8 blocks of [128,128] -> xT_t[d][128, 128]
                xT_t = []
                for d in range(ND):
                    pt = xtp.tile([128, 128], F32, tag='xtp')
                    nc.tensor.transpose(pt[:], x_t[:, bass.ts(d, 128)], ident[:])
                    st = xts.tile([128, 128], F32R, tag=f'xts{d}')
                    nc.scalar.activation(st[:], pt[:],
                                         mybir.ActivationFunctionType.Copy)
                    xT_t.append(st)
                # qkv matmuls: psum [128, 512] per feature block
                qk_sb = qkvs.tile([128, 2 * DIM], F32, tag='qkv')
                v_sb = qkvs.tile([128, DIM], F32R, tag='vsb')
                for nb in range(6):
                    bq_t = bqs.tile([128, 512], F32, tag='bq')
                    nc.sync.dma_start(bq_t[:], bqkv_e.ap()[:, bass.ts(nb, 512)])
                    ps = qkvp.tile([128, 512], F32, tag='ps')
                    for d in range(ND):
                        nc.tensor.matmul(ps[:], xT_t[d][:],
                                         w_sb[d][:, bass.ts(nb, 512)],
                                         start=(d == 0), stop=(d == ND - 1))
                    if nb < 4:
                        nc.vector.tensor_add(qk_sb[:, bass.ts(nb, 512)], ps[:],
                                             bq_t[:])
                    else:
                        nc.vector.tensor_add(v_sb[:, bass.ts(nb - 4, 512)],
                                             ps[:], bq_t[:])
                # views [128, H, HD]
                q3 = qk_sb[:, 0:DIM].rearrange('p (h e) -> p h e', h=H)
                k3 = qk_sb[:, DIM:2 * DIM].rearrange('p (h e) -> p h e', h=H)
                # --- rmsnorm ---
                sq = nrm.tile([128, H, HD], F32, tag='sq')
                ss_q = nrm.tile([128, H], F32, tag='ssq')
                ss_k = nrm.tile([128, H], F32, tag='ssk')
                rq = nrm.tile([128, H], F32, tag='rq')
                rk = nrm.tile([128, H], F32, tag='rk')
                nc.vector.tensor_mul(sq[:], q3, q3)
                nc.vector.tensor_reduce(ss_q[:], sq[:], mybir.AxisListType.X,
                                        mybir.AluOpType.add)
                nc.vector.tensor_mul(sq[:], k3, k3)
                nc.vector.tensor_reduce(ss_k[:], sq[:], mybir.AxisListType.X,
                                        mybir.AluOpType.add)
                # rstd_q (with 1/sqrt(HD) folded) = sqrt(1/(ss + 64eps))
                nc.vector.tensor_scalar_add(ss_q[:], ss_q[:], 64.0 * EPS)
                nc.vector.reciprocal(ss_q[:], ss_q[:])
                nc.scalar.activation(rq[:], ss_q[:],
                                     mybir.ActivationFunctionType.Sqrt)
                # rstd_k = sqrt(64/(ss + 64eps))
                nc.vector.tensor_scalar_add(ss_k[:], ss_k[:], 64.0 * EPS)
                nc.vector.reciprocal(ss_k[:], ss_k[:])
                nc.scalar.activation(rk[:], ss_k[:],
                                     mybir.ActivationFunctionType.Sqrt, scale=64.0)
                for h in range(H):
                    nc.vector.tensor_scalar_mul(q3[:, h, :], q3[:, h, :],
                                                rq[:, h:h + 1])
                    nc.vector.tensor_scalar_mul(k3[:, h, :], k3[:, h, :],
                                                rk[:, h:h + 1])
                nc.vector.tensor_mul(q3, q3, qn_w[:])
                nc.vector.tensor_mul(k3, k3, kn_w[:])
                # --- rope ---
                cos_t = cspool.tile([128, H, HD], F32, tag='cos')
                sin_t = cspool.tile([128, H, HD], F32, tag='sin')
                nc.sync.dma_start(cos_t[:], cos_e.ap()[pos_tile])
                nc.sync.dma_start(sin_t[:], sin_e.ap()[pos_tile])
                qr = rot.tile([128, H, HD], F32, tag='qr')
                kr = rot.tile([128, H, HD], F32, tag='kr')
                for (src, m1, ttag) in ((q3, qr, 'qt'), (k3, kr, 'kt')):
                    t2 = rot.tile([128, H, ROPE_DIM], F32, tag=ttag)
                    nc.vector.tensor_mul(m1[:], src, cos_t[:])
                    nc.vector.tensor_mul(t2[:], src[:, :, ROPE_DIM:],
                                         sin_t[:, :, 0:ROPE_DIM])
                    nc.vector.tensor_sub(m1[:, :, 0:ROPE_DIM],
                                         m1[:, :, 0:ROPE_DIM], t2[:])
                    nc.vector.tensor_mul(t2[:], src[:, :, 0:ROPE_DIM],
                                         sin_t[:, :, ROPE_DIM:])
                    nc.vector.tensor_add(m1[:, :, ROPE_DIM:],
                                         m1[:, :, ROPE_DIM:], t2[:])
                # --- transpose per head into qkT_d; v straight to DRAM ---
                for h in range(H):
                    for (mat, base) in ((qr, 0), (kr, DIM)):
                        pt = htp.tile([64, 128], F32, tag='htp')
                        nc.tensor.transpose(pt[:], mat[:, h, :], ident[:])
                        st = hts.tile([64, 128], F32R, tag='hts')
                        nc.scalar.activation(st[:], pt[:],
                                             mybir.ActivationFunctionType.Copy)
                        nc.gpsimd.dma_start(
                            qkT_d[base + h * HD:base + (h + 1) * HD,
                                  bass.ts(it, 128)], st[:])
                nc.gpsimd.dma_start(v_d[bass.ts(it, 128), :], v_sb[:])

        # ---- Phase 2: attention per (batch, head) ----
        with tc.tile_pool(name='kq', bufs=2) as kq, \
             tc.tile_pool(name='vext', bufs=2) as vext, \
             tc.tile_pool(name='sp', bufs=2, space='PSUM') as sp, \
             tc.tile_pool(name='pt', bufs=3) as ptp, \
             tc.tile_pool(name='pvp', bufs=2, space='PSUM') as pvp, \
             tc.tile_pool(name='bcp', bufs=2, space='PSUM') as bcp, \
             tc.tile_pool(name='rz', bufs=2) as rzp, \
             tc.tile_pool(name='ao', bufs=3) as aop:
            for b in range(BL):
                for h in range(H):
                    kT_h = kq.tile([64, S], F32R, tag='k')
                    qT_h = kq.tile([64, S], F32R, tag='q')
                    nc.sync.dma_start(
                        kT_h[:], qkT_d[DIM + h * HD:DIM + (h + 1) * HD,
                                       bass.ts(b, S)])
                    nc.sync.dma_start(
                        qT_h[:], qkT_d[h * HD:(h + 1) * HD, bass.ts(b, S)])
                    v_t = []
                    for j in range(8):
                        vt = vext.tile([128, HD + 1], F32R, tag=f'v{j}')
                        nc.sync.dma_start(
                            vt[:, 0:HD],
                            v_d[b * S + j * 128:b * S + (j + 1) * 128,
                                h * HD:(h + 1) * HD])
                        nc.scalar.activation(vt[:, HD:HD + 1], ones_col[:],
                                             mybir.ActivationFunctionType.Copy)
                        v_t.append(vt)
                    for ic in range(2):
                        pv = pvp.tile([HD + 1, 512], F32, tag='pv')
                        for j in range(8):
                            ps = sp.tile([128, 512], F32, tag='sp')
                            nc.tensor.matmul(
                                ps[:], kT_h[:, bass.ts(j, 128)],
                                qT_h[:, bass.ts(ic, 512)],
                                start=True, stop=True)
                            pt = ptp.tile([128, 512], F32R, tag='pt')
                            nc.scalar.activation(
                                pt[:], ps[:], mybir.ActivationFunctionType.Exp)
                            nc.tensor.matmul(pv[:], v_t[j][:], pt[:],
                                             start=(j == 0), stop=(j == 7))
                        rz = rzp.tile([1, 512], F32R, tag='rz')
                        nc.vector.reciprocal(rz[:], pv[64:65, :])
                        bc = bcp.tile([64, 512], F32, tag='bc')
                        nc.tensor.matmul(bc[:], ones_row[:], rz[:],
                                         start=True, stop=True)
                        ao = aop.tile([64, 512], F32R, tag='ao')
                        nc.scalar.activation(ao[:], pv[0:64, :],
                                             mybir.ActivationFunctionType.Copy)
                        nc.vector.tensor_mul(ao[:], ao[:], bc[:])
                        nc.gpsimd.dma_start(
                            attnT_d[h * HD:(h + 1) * HD,
                                    b * S + ic * 512:b * S + (ic + 1) * 512],
                            ao[:])

        # ---- Phase 3: out projection ----
        with tc.tile_pool(name='wo', bufs=1) as wo, \
             tc.tile_pool(name='at', bufs=2) as atp, \
             tc.tile_pool(name='yp', bufs=2, space='PSUM') as yp, \
             tc.tile_pool(name='ys', bufs=2) as ys:
            wo_sb = []
            for d in range(ND):
                wt = wo.tile([128, DIM], F32R, tag=f'wo{d}')
                nc.sync.dma_start(wt[:], wout_e.ap()[bass.ts(d, 128), :])
                wo_sb.append(wt)
            bo_sb = wo.tile([128, DIM], F32, tag='bo')
            nc.sync.dma_start(bo_sb[:], bout_e.ap()[:])
            for it in range(NT):
                a_t = []
                for d in range(ND):
                    at = atp.tile([128, 128], F32R, tag=f'at{d}')
                    nc.sync.dma_start(at[:],
                                      attnT_d[bass.ts(d, 128), bass.ts(it, 128)])
                    a_t.append(at)
                y_sb = ys.tile([128, DIM], F32, tag='y')
                for nb in range(2):
                    ps = yp.tile([128, 512], F32, tag='yp')
                    for d in range(ND):
                        nc.tensor.matmul(ps[:], a_t[d][:],
                                         wo_sb[d][:, bass.ts(nb, 512)],
                                         start=(d == 0), stop=(d == ND - 1))
                    nc.vector.tensor_add(y_sb[:, bass.ts(nb, 512)], ps[:],
                                         bo_sb[:, bass.ts(nb, 512)])
                nc.sync.dma_start(out_ap[bass.ts(it, 128), :], y_sb[:])
    nc.compile()
    return nc


_NC_CACHE = None
TRACE = False
LAST_RESULT = None


def kernel(x, w_qkv, b_qkv, q_norm_w, k_norm_w, w_out, b_out):
    global _NC_CACHE
    if _NC_CACHE is None:
        _NC_CACHE = build_graph()
    nc = _NC_CACHE
    cos, sin = _rope_tables()                       # [S, HD]
    cos16 = np.broadcast_to(cos.reshape(S // 128, 128, 1, HD),
                            (S // 128, 128, H, HD)).copy()
    sin16 = np.broadcast_to(sin.reshape(S // 128, 128, 1, HD),
                            (S // 128, 128, H, HD)).copy()
    bqkv_t = np.broadcast_to(b_qkv[None, :], (128, 3 * DIM)).copy()
    bout_t = np.broadcast_to(b_out[None, :], (128, DIM)).copy()
    qnw_t = np.broadcast_to(q_norm_w[None, None, :], (128, H, HD)).copy()
    knw_t = np.broadcast_to(k_norm_w[None, None, :], (128, H, HD)).copy()
    ident = np.eye(128, dtype=np.float32)
    x = np.ascontiguousarray(x, dtype=np.float32)
    in_maps = []
    for c in range(NCORES):
        in_maps.append({
            'x': np.ascontiguousarray(x[c * BL:(c + 1) * BL].reshape(T, DIM)),
            'w_qkv': np.ascontiguousarray(w_qkv, dtype=np.float32),
            'w_out': np.ascontiguousarray(w_out, dtype=np.float32),
            'b_qkv_t': bqkv_t, 'b_out_t': bout_t,
            'cos16': cos16, 'sin16': sin16, 'ident': ident,
            'qn_w': qnw_t, 'kn_w': knw_t,
        })
    global LAST_RESULT
    res = run_bass_kernel_spmd(nc, in_maps, core_ids=list(range(NCORES)),
                               trace=TRACE)
    LAST_RESULT = res
    outs = [res.results[c]['out'].reshape(BL, S, DIM) for c in range(NCORES)]
    return np.concatenate(outs, axis=0)



# revision 5
# speedup vs baseline: 1.5160x; 1.5160x over previous
"""Trainium2 distributed attention kernel for nn_Attention_72095321030782.

B=16, S=1024, DIM=1024, H=16, HD=64. Batch data-parallel over 8 cores
(2 batches/core), no collectives. Per core, per batch (fully
SBUF-resident intermediates, bf16 matmul operands):
  P1: x tile -> bf16 -> DMA-xbar transpose -> QKV matmul (bf16 weights)
      -> QK RMSNorm (DVE, free-dim-broadcast rstd) + 2D RoPE (bf16)
      -> DMA-xbar transpose q/k into [feat, tok] layout; v + ones column.
  P2: per (b,h): S^T = k.q via PE, exp on ACT (3-bank groups, bf16 out),
      PV accumulate with ones-column denominator, normalize via
      ones-matmul broadcast, attnT chunks -> DRAM.
  P3: out = attnT.T @ w_out + b_out, streamed from DRAM.
"""

import math
from contextlib import ExitStack

import numpy as np
import ml_dtypes

import concourse.bass as bass
import concourse.tile as tile
from concourse import bacc, mybir
from concourse.bass_utils import run_bass_kernel_spmd

B, S, DIM, H = 16, 1024, 1024, 16
HD = DIM // H            # 64
RD = HD // 2             # 32 rope halves
FT, PT_LEN = 32, 16
THETA = 10000.0
EPS = 1e-6
NCORES = 8
BL = B // NCORES         # 2 batches per core
T = BL * S               # 2048 tokens per core
TPB = S // 128           # 8 token tiles per batch
PB = H // 2              # 8 head-pair blocks
F32 = mybir.dt.float32
BF16 = mybir.dt.bfloat16
AF = mybir.ActivationFunctionType


def _rope_tables():
    freqs = 1.0 / THETA ** (np.arange(0, RD, 2, dtype=np.float32) / RD)
    t = np.arange(FT, dtype=np.float32) / FT * PT_LEN
    fs = np.einsum('n,f->nf', t, freqs).astype(np.float32)
    fs = np.repeat(fs, 2, axis=-1)                       # [FT, 32]
    fh = np.broadcast_to(fs[:, None, :], (FT, FT, RD))
    fw = np.broadcast_to(fs[None, :, :], (FT, FT, RD))
    f = np.concatenate([fh, fw], axis=-1).reshape(S, HD)
    return np.cos(f).astype(np.float32), np.sin(f).astype(np.float32)


def build_graph():
    nc = bacc.Bacc('TRN2', target_bir_lowering=False, debug=False,
                   num_devices=NCORES)
    x_e = nc.declare_dram_parameter('x', [T, DIM], F32, isOutput=False)
    wq_e = nc.declare_dram_parameter('wq_b', [DIM, 3 * DIM], BF16, isOutput=False)
    wo_e = nc.declare_dram_parameter('wo_b', [DIM, DIM], BF16, isOutput=False)
    bq_e = nc.declare_dram_parameter('bq_b', [128, 3 * DIM], BF16, isOutput=False)
    bo_e = nc.declare_dram_parameter('bo_b', [128, DIM], BF16, isOutput=False)
    # cos/sinF tables: [128, TPB, 8*HD] (8-head replicated, sinF sign-folded)
    cos_e = nc.declare_dram_parameter('cos_b', [128, TPB, 8 * HD], BF16,
                                      isOutput=False)
    sin_e = nc.declare_dram_parameter('sinF_b', [128, TPB, 8 * HD], BF16,
                                      isOutput=False)
    qnw_e = nc.declare_dram_parameter('qnw_b', [128, DIM], BF16, isOutput=False)
    knw_e = nc.declare_dram_parameter('knw_b', [128, DIM], BF16, isOutput=False)
    out_e = nc.declare_dram_parameter('out', [T, DIM], F32, isOutput=True)

    x_ap = x_e.ap()
    out_ap = out_e.ap()

    with nc.allow_low_precision(reason='bf16 matmul pipeline'), \
         tile.TileContext(nc) as tc, ExitStack() as ctx:
        dram = ctx.enter_context(tc.tile_pool(name='dram', bufs=1, space='DRAM'))
        # attnT per batch: [pb][it][128 feat, 128 tok] blocked
        attnT_d = dram.tile([BL, PB, TPB, 128, 128], BF16)

        const = ctx.enter_context(tc.tile_pool(name='const', bufs=1))
        wq_sb = []
        for d in range(8):
            wt = const.tile([128, 3 * DIM], BF16, tag=f'wq{d}')
            nc.sync.dma_start(wt[:], wq_e.ap()[bass.ts(d, 128), :])
            wq_sb.append(wt)
        wo_sb = []
        for d in range(8):
            wt = const.tile([128, DIM], BF16, tag=f'wo{d}')
            nc.sync.dma_start(wt[:], wo_e.ap()[bass.ts(d, 128), :])
            wo_sb.append(wt)
        bq_sb = const.tile([128, 3 * DIM], BF16)
        nc.sync.dma_start(bq_sb[:], bq_e.ap()[:])
        bo_sb = const.tile([128, DIM], BF16)
        nc.sync.dma_start(bo_sb[:], bo_e.ap()[:])
        cos_sb = const.tile([128, TPB, 8 * HD], BF16)
        nc.sync.dma_start(cos_sb[:], cos_e.ap()[:])
        sin_sb = const.tile([128, TPB, 8 * HD], BF16)
        nc.sync.dma_start(sin_sb[:], sin_e.ap()[:])
        qnw_sb = const.tile([128, DIM], BF16)
        nc.sync.dma_start(qnw_sb[:], qnw_e.ap()[:])
        knw_sb = const.tile([128, DIM], BF16)
        nc.sync.dma_start(knw_sb[:], knw_e.ap()[:])
        ones_f = const.tile([1, HD], F32)
        nc.vector.memset(ones_f[:], 1.0)
        ones_b = const.tile([1, HD], BF16)
        nc.vector.tensor_copy(ones_b[:], ones_f[:])

        # per-batch resident q/k transposed + v (rotate via bufs=1 tags)
        res = ctx.enter_context(tc.tile_pool(name='res', bufs=1))

        for b in range(BL):
            qT_all = res.tile([128, PB, TPB, 128], BF16, tag='qT')
            kT_all = res.tile([128, PB, TPB, 128], BF16, tag='kT')
            v_all = res.tile([128, TPB, H, HD + 1], BF16, tag='v')

            # ---- P1: QKV + norm + rope for this batch ----
            with tc.tile_pool(name='xin', bufs=2) as xin, \
                 tc.tile_pool(name='xbp', bufs=2) as xbp, \
                 tc.tile_pool(name='xtp', bufs=2) as xtp, \
                 tc.tile_pool(name='qkvp', bufs=3, space='PSUM') as qkvp, \
                 tc.tile_pool(name='raw', bufs=2) as rawp, \
                 tc.tile_pool(name='sqp', bufs=2) as sqp, \
                 tc.tile_pool(name='stp', bufs=2) as stp, \
                 tc.tile_pool(name='nrm', bufs=2) as nrmp, \
                 tc.tile_pool(name='rop', bufs=2) as ropp, \
                 tc.tile_pool(name='ttp', bufs=2) as ttp:
                nc.vector.memset(v_all[:, :, :, HD:HD + 1], 1.0)
                for it in range(TPB):
                    tok0 = b * S + it * 128
                    x_t = xin.tile([128, DIM], F32, tag='x')
                    nc.sync.dma_start(x_t[:], x_ap[tok0:tok0 + 128, :])
                    xb = xbp.tile([128, DIM], BF16, tag='xb')
                    nc.scalar.activation(xb[:], x_t[:], AF.Copy)
                    xT = xtp.tile([128, 8, 128], BF16, tag='xT')
                    nc.sync.dma_start_transpose(xT[:], xb[:])
                    # qkv: 6 psum blocks of 512
                    qraw = rawp.tile([128, DIM], BF16, tag='qraw')
                    kraw = rawp.tile([128, DIM], BF16, tag='kraw')
                    dsts = [(qraw, 0), (qraw, 512), (kraw, 0), (kraw, 512)]
                    for nb in range(6):
                        ps = qkvp.tile([128, 512], F32, tag='ps')
                        for d in range(8):
                            nc.tensor.matmul(ps[:], xT[:, d, :],
                                             wq_sb[d][:, bass.ts(nb, 512)],
                                             start=(d == 0), stop=(d == 7))
                        if nb < 4:
                            dst, off = dsts[nb]
                            nc.vector.tensor_add(dst[:, off:off + 512], ps[:],
                                                 bq_sb[:, bass.ts(nb, 512)])
                        else:
                            # v: heads (nb-4)*8 .. +8, strided into v_all
                            h0 = (nb - 4) * 8
                            vd = v_all[:, it, h0:h0 + 8, 0:HD]
                            nc.vector.tensor_add(
                                vd, ps[:].rearrange('p (h e) -> p h e', h=8),
                                bq_sb[:, bass.ts(nb, 512)]
                                .rearrange('p (h e) -> p h e', h=8))
                    # rmsnorm: ss = sum over HD of raw^2 (per head)
                    for (raw, nw, kscale, rtag) in (
                            (qraw, qnw_sb, 1.0, 'q'), (kraw, knw_sb, 64.0, 'k')):
                        sq = sqp.tile([128, DIM], BF16, tag='sq')
                        nc.vector.tensor_mul(sq[:], raw[:], raw[:])
                        ss = stp.tile([128, H], F32, tag=f'ss{rtag}')
                        nc.vector.tensor_reduce(
                            ss[:], sq[:].rearrange('p (h e) -> p h e', h=H),
                            mybir.AxisListType.X, mybir.AluOpType.add)
                        nc.vector.tensor_scalar_add(ss[:], ss[:], HD * EPS)
                        nc.vector.reciprocal(ss[:], ss[:])
                        rs = stp.tile([128, H], F32, tag=f'rs{rtag}')
                        nc.scalar.activation(rs[:], ss[:], AF.Sqrt, scale=kscale)
                        # normed = raw * rs (free-bcast) * norm_w -> bf16
                        nn = nrmp.tile([128, DIM], BF16, tag=f'nn{rtag}')
                        rsv = rs[:].unsqueeze(2).broadcast_to([128, H, HD])
                        nc.vector.tensor_mul(
                            nn[:].rearrange('p (h e) -> p h e', h=H),
                            raw[:].rearrange('p (h e) -> p h e', h=H), rsv)
                        nc.vector.tensor_mul(nn[:], nn[:], nw[:])
                        # rope
                        n3 = nn[:].rearrange('p (h e) -> p h e', h=H)
                        ro = ropp.tile([128, DIM], BF16, tag=f'ro{rtag}')
                        r3 = ro[:].rearrange('p (h e) -> p h e', h=H)
                        t2 = ropp.tile([128, DIM], BF16, tag=f't2{rtag}')
                        t3 = t2[:].rearrange('p (h e) -> p h e', h=H)
                        cosv = cos_sb[:, it, :].rearrange('p (h e) -> p h e', h=8)
                        sinv = sin_sb[:, it, :].rearrange('p (h e) -> p h e', h=8)
                        for hh in (slice(0, 8), slice(8, 16)):
                            nc.vector.tensor_mul(r3[:, hh, :], n3[:, hh, :], cosv)
                            nc.vector.tensor_mul(t3[:, hh, 0:RD],
                                                 n3[:, hh, RD:HD],
                                                 sinv[:, :, 0:RD])
                            nc.vector.tensor_mul(t3[:, hh, RD:HD],
                                                 n3[:, hh, 0:RD],
                                                 sinv[:, :, RD:HD])
                        nc.vector.tensor_add(ro[:], ro[:], t2[:])
                        # transpose to [feat, tok] and relayout
                        tt = ttp.tile([128, 8, 128], BF16, tag=f'tt{rtag}')
                        nc.sync.dma_start_transpose(tt[:], ro[:])
                        dstT = qT_all if rtag == 'q' else kT_all
                        nc.vector.tensor_copy(dstT[:, :, it, :], tt[:])

            # ---- P2: attention for this batch ----
            with tc.tile_pool(name='sps', bufs=2, space='PSUM') as spsp, \
                 tc.tile_pool(name='pvp', bufs=1, space='PSUM') as pvp, \
                 tc.tile_pool(name='bcp', bufs=1, space='PSUM') as bcp, \
                 tc.tile_pool(name='ptp', bufs=2) as ptp, \
                 tc.tile_pool(name='rzp', bufs=2) as rzp, \
                 tc.tile_pool(name='aop', bufs=2) as aop:
                for h in range(H):
                    pb, off = h >> 1, (h & 1) * 64
                    kT_h = kT_all[off:off + 64, pb, :, :]   # [64, 8, 128]
                    qT_h = qT_all[off:off + 64, pb, :, :]
                    for ic in range(2):
                        rhs = qT_h[:, ic * 4:(ic + 1) * 4, :]  # [64, 4, 128]
                        pv = pvp.tile([HD + 1, 512], F32, tag='pv')
                        for jg in ((0, 1, 2), (3, 4, 5), (6, 7)):
                            w = len(jg) * 512
                            sps = spsp.tile([128, 1536], F32, tag='sps')
                            for i, j in enumerate(jg):
                                nc.tensor.matmul(
                                    sps[:, bass.ts(i, 512)], kT_h[:, j, :],
                                    rhs, start=True, stop=True)
                            pt = ptp.tile([128, 1536], BF16, tag='pt')
                            nc.scalar.activation(pt[:, 0:w], sps[:, 0:w], AF.Exp)
                            for i, j in enumerate(jg):
                                nc.tensor.matmul(
                                    pv[:], v_all[:, j, h, :],
                                    pt[:, bass.ts(i, 512)],
                                    start=(j == 0), stop=(j == 7))
                        rz = rzp.tile([1, 512], F32, tag='rz')
                        nc.vector.reciprocal(rz[:], pv[HD:HD + 1, :])
                        rzb = rzp.tile([1, 512], BF16, tag='rzb')
                        nc.vector.tensor_copy(rzb[:], rz[:])
                        bc = bcp.tile([HD, 512], F32, tag='bc')
                        nc.tensor.matmul(bc[:], ones_b[:], rzb[:],
                                         start=True, stop=True)
                        an = aop.tile([HD, 512], BF16, tag='an')
                        nc.vector.tensor_copy(an[:], pv[0:HD, :])
                        ao = aop.tile([HD, 512], BF16, tag='ao')
                        nc.vector.tensor_mul(ao[:], an[:], bc[:])
                        dst = attnT_d[b, pb, ic * 4:(ic + 1) * 4,
                                      off:off + 64, :].transpose([1, 0, 2])
                        nc.gpsimd.dma_start(
                            dst, ao[:].rearrange('p (i t) -> p i t', i=4))

            # ---- P3: out projection for this batch ----
            with tc.tile_pool(name='atp', bufs=3) as atp, \
                 tc.tile_pool(name='yps', bufs=2, space='PSUM') as yps, \
                 tc.tile_pool(name='ysb', bufs=2) as ysb:
                for it in range(TPB):
                    a_t = []
                    for d in range(8):
                        at = atp.tile([128, 128], BF16, tag=f'at{d}')
                        nc.sync.dma_start(at[:], attnT_d[b, d, it, :, :])
                        a_t.append(at)
                    y = ysb.tile([128, DIM], F32, tag='y')
                    for nb in range(2):
                        ps = yps.tile([128, 512], F32, tag='yp')
                        for d in range(8):
                            nc.tensor.matmul(ps[:], a_t[d][:],
                                             wo_sb[d][:, bass.ts(nb, 512)],
                                             start=(d == 0), stop=(d == 7))
                        nc.vector.tensor_add(y[:, bass.ts(nb, 512)], ps[:],
                                             bo_sb[:, bass.ts(nb, 512)])
                    nc.gpsimd.dma_start(
                        out_ap[b * S + it * 128:b * S + (it + 1) * 128, :], y[:])
    nc.compile()
    return nc


_NC_CACHE = None
TRACE = False
LAST_RESULT = None


def kernel(x, w_qkv, b_qkv, q_norm_w, k_norm_w, w_out, b_out):
    global _NC_CACHE, LAST_RESULT
    if _NC_CACHE is None:
        _NC_CACHE = build_graph()
    nc = _NC_CACHE
    bf = ml_dtypes.bfloat16
    cos, sin = _rope_tables()                       # [S, HD] f32
    # tables: [128 part, TPB pos-tiles, 8 heads * HD], sinF sign-folded
    cos_t = cos.reshape(TPB, 128, HD).transpose(1, 0, 2)      # [128, TPB, HD]
    sinF = sin.copy()
    sinF[:, 0:RD] = -sinF[:, 0:RD]
    sin_t = sinF.reshape(TPB, 128, HD).transpose(1, 0, 2)
    cos_b = np.broadcast_to(cos_t[:, :, None, :],
                            (128, TPB, 8, HD)).reshape(128, TPB, 8 * HD)
    sin_b = np.broadcast_to(sin_t[:, :, None, :],
                            (128, TPB, 8, HD)).reshape(128, TPB, 8 * HD)
    com = {
        'wq_b': np.ascontiguousarray(w_qkv, dtype=np.float32).astype(bf),
        'wo_b': np.ascontiguousarray(w_out, dtype=np.float32).astype(bf),
        'bq_b': np.broadcast_to(b_qkv[None, :].astype(np.float32),
                                (128, 3 * DIM)).astype(bf),
        'bo_b': np.broadcast_to(b_out[None, :].astype(np.float32),
                                (128, DIM)).astype(bf),
        'cos_b': np.ascontiguousarray(cos_b).astype(bf),
        'sinF_b': np.ascontiguousarray(sin_b).astype(bf),
        'qnw_b': np.broadcast_to(np.tile(q_norm_w.astype(np.float32), H)[None, :],
                                 (128, DIM)).astype(bf),
        'knw_b': np.broadcast_to(np.tile(k_norm_w.astype(np.float32), H)[None, :],
                                 (128, DIM)).astype(bf),
    }
    x = np.ascontiguousarray(x, dtype=np.float32)
    in_maps = []
    for c in range(NCORES):
        m = dict(com)
        m['x'] = np.ascontiguousarray(x[c * BL:(c + 1) * BL].reshape(T, DIM))
        in_maps.append(m)
    res = run_bass_kernel_spmd(nc, in_maps, core_ids=list(range(NCORES)),
                               trace=TRACE)
    LAST_RESULT = res
    outs = [res.results[c]['out'].reshape(BL, S, DIM) for c in range(NCORES)]
    return np.concatenate(outs, axis=0)


# revision 6
# speedup vs baseline: 2.0336x; 1.3414x over previous
"""Trainium2 distributed attention kernel for nn_Attention_72095321030782.

B=16, S=1024, DIM=1024, H=16, HD=64. Batch data-parallel over 8 cores
(2 batches/core), no collectives. Per core, per batch (fully
SBUF-resident intermediates, bf16 matmul operands):
  P1: x tile -> bf16 -> DMA-xbar transpose -> QKV matmul (bf16 weights)
      -> QK RMSNorm (DVE, free-dim-broadcast rstd) + 2D RoPE (bf16)
      -> DMA-xbar transpose q/k into [feat, tok] layout; v + ones column.
  P2: per (b,h): S^T = k.q via PE, exp on ACT (3-bank groups, bf16 out),
      PV accumulate with ones-column denominator, normalize via
      ones-matmul broadcast, attnT chunks -> DRAM.
  P3: out = attnT.T @ w_out + b_out, streamed from DRAM.
"""

import math
from contextlib import ExitStack

import numpy as np
import ml_dtypes

import concourse.bass as bass
import concourse.tile as tile
from concourse import bacc, mybir
from concourse.bass_utils import run_bass_kernel_spmd

B, S, DIM, H = 16, 1024, 1024, 16
HD = DIM // H            # 64
RD = HD // 2             # 32 rope halves
FT, PT_LEN = 32, 16
THETA = 10000.0
EPS = 1e-6
NCORES = 8
BL = B // NCORES         # 2 batches per core
T = BL * S               # 2048 tokens per core
TPB = S // 128           # 8 token tiles per batch
PB = H // 2              # 8 head-pair blocks
F32 = mybir.dt.float32
BF16 = mybir.dt.bfloat16
AF = mybir.ActivationFunctionType


def _rope_tables():
    freqs = 1.0 / THETA ** (np.arange(0, RD, 2, dtype=np.float32) / RD)
    t = np.arange(FT, dtype=np.float32) / FT * PT_LEN
    fs = np.einsum('n,f->nf', t, freqs).astype(np.float32)
    fs = np.repeat(fs, 2, axis=-1)                       # [FT, 32]
    fh = np.broadcast_to(fs[:, None, :], (FT, FT, RD))
    fw = np.broadcast_to(fs[None, :, :], (FT, FT, RD))
    f = np.concatenate([fh, fw], axis=-1).reshape(S, HD)
    return np.cos(f).astype(np.float32), np.sin(f).astype(np.float32)


def build_graph():
    nc = bacc.Bacc('TRN2', target_bir_lowering=False, debug=False,
                   num_devices=NCORES)
    x_e = nc.declare_dram_parameter('x', [T, DIM], F32, isOutput=False)
    wq_e = nc.declare_dram_parameter('wq_b', [DIM, 3 * DIM], BF16, isOutput=False)
    wo_e = nc.declare_dram_parameter('wo_b', [DIM, DIM], BF16, isOutput=False)
    bq_e = nc.declare_dram_parameter('bq_b', [128, 3 * DIM], BF16, isOutput=False)
    bo_e = nc.declare_dram_parameter('bo_b', [128, DIM], BF16, isOutput=False)
    # cos/sinF tables: [128, TPB, 8*HD] (8-head replicated, sinF sign-folded)
    cos_e = nc.declare_dram_parameter('cos_b', [128, TPB, 8 * HD], BF16,
                                      isOutput=False)
    sin_e = nc.declare_dram_parameter('sinF_b', [128, TPB, 8 * HD], BF16,
                                      isOutput=False)
    qnw_e = nc.declare_dram_parameter('qnw_b', [128, DIM], BF16, isOutput=False)
    knw_e = nc.declare_dram_parameter('knw_b', [128, DIM], BF16, isOutput=False)
    out_e = nc.declare_dram_parameter('out', [T, DIM], F32, isOutput=True)

    x_ap = x_e.ap()
    out_ap = out_e.ap()

    with nc.allow_low_precision(reason='bf16 matmul pipeline'), \
         tile.TileContext(nc) as tc, ExitStack() as ctx:
        dram = ctx.enter_context(tc.tile_pool(name='dram', bufs=1, space='DRAM'))
        # attnT per batch: [pb][it][128 feat, 128 tok] blocked
        attnT_d = dram.tile([BL, PB, TPB, 128, 128], BF16)

        const = ctx.enter_context(tc.tile_pool(name='const', bufs=1))
        wq_sb = []
        for d in range(8):
            wt = const.tile([128, 3 * DIM], BF16, tag=f'wq{d}')
            nc.sync.dma_start(wt[:], wq_e.ap()[bass.ts(d, 128), :])
            wq_sb.append(wt)
        wo_sb = []
        for d in range(8):
            wt = const.tile([128, DIM], BF16, tag=f'wo{d}')
            nc.sync.dma_start(wt[:], wo_e.ap()[bass.ts(d, 128), :])
            wo_sb.append(wt)
        bq_sb = const.tile([128, 3 * DIM], BF16)
        nc.sync.dma_start(bq_sb[:], bq_e.ap()[:])
        bo_sb = const.tile([128, DIM], BF16)
        nc.sync.dma_start(bo_sb[:], bo_e.ap()[:])
        cos_sb = const.tile([128, TPB, 8 * HD], BF16)
        nc.sync.dma_start(cos_sb[:], cos_e.ap()[:])
        sin_sb = const.tile([128, TPB, 8 * HD], BF16)
        nc.sync.dma_start(sin_sb[:], sin_e.ap()[:])
        qnw_sb = const.tile([128, DIM], BF16)
        nc.sync.dma_start(qnw_sb[:], qnw_e.ap()[:])
        knw_sb = const.tile([128, DIM], BF16)
        nc.sync.dma_start(knw_sb[:], knw_e.ap()[:])
        ones_f = const.tile([1, HD], F32)
        nc.vector.memset(ones_f[:], 1.0)
        ones_b = const.tile([1, HD], BF16)
        nc.vector.tensor_copy(ones_b[:], ones_f[:])

        # per-batch resident q/k transposed + v (rotate via bufs=1 tags)
        res = ctx.enter_context(tc.tile_pool(name='res', bufs=1))

        for b in range(BL):
            qT_all = res.tile([128, PB, TPB, 128], BF16, tag='qT')
            kT_all = res.tile([128, PB, TPB, 128], BF16, tag='kT')
            v_all = res.tile([128, TPB, H, HD + 1], BF16, tag='v')

            # ---- P1: QKV + norm + rope for this batch ----
            with tc.tile_pool(name='xin', bufs=2) as xin, \
                 tc.tile_pool(name='xbp', bufs=2) as xbp, \
                 tc.tile_pool(name='xtp', bufs=2) as xtp, \
                 tc.tile_pool(name='qkvp', bufs=3, space='PSUM') as qkvp, \
                 tc.tile_pool(name='raw', bufs=2) as rawp, \
                 tc.tile_pool(name='sqp', bufs=2) as sqp, \
                 tc.tile_pool(name='stp', bufs=2) as stp, \
                 tc.tile_pool(name='nrm', bufs=2) as nrmp, \
                 tc.tile_pool(name='rop', bufs=2) as ropp, \
                 tc.tile_pool(name='ttp', bufs=2) as ttp:
                nc.vector.memset(v_all[:, :, :, HD:HD + 1], 1.0)
                for it in range(TPB):
                    tok0 = b * S + it * 128
                    x_t = xin.tile([128, DIM], F32, tag='x')
                    nc.sync.dma_start(x_t[:], x_ap[tok0:tok0 + 128, :])
                    xb = xbp.tile([128, DIM], BF16, tag='xb')
                    nc.scalar.activation(xb[:], x_t[:], AF.Copy)
                    xT = xtp.tile([128, 8, 128], BF16, tag='xT')
                    nc.sync.dma_start_transpose(xT[:], xb[:])
                    # qkv: 6 psum blocks of 512
                    qraw = rawp.tile([128, DIM], BF16, tag='qraw')
                    kraw = rawp.tile([128, DIM], BF16, tag='kraw')
                    dsts = [(qraw, 0), (qraw, 512), (kraw, 0), (kraw, 512)]
                    for nb in range(6):
                        ps = qkvp.tile([128, 512], F32, tag='ps')
                        for d in range(8):
                            nc.tensor.matmul(ps[:], xT[:, d, :],
                                             wq_sb[d][:, bass.ts(nb, 512)],
                                             start=(d == 0), stop=(d == 7))
                        if nb < 4:
                            dst, off = dsts[nb]
                            nc.vector.tensor_add(dst[:, off:off + 512], ps[:],
                                                 bq_sb[:, bass.ts(nb, 512)])
                        else:
                            # v: heads (nb-4)*8 .. +8, strided into v_all
                            h0 = (nb - 4) * 8
                            vd = v_all[:, it, h0:h0 + 8, 0:HD]
                            nc.vector.tensor_add(
                                vd, ps[:].rearrange('p (h e) -> p h e', h=8),
                                bq_sb[:, bass.ts(nb, 512)]
                                .rearrange('p (h e) -> p h e', h=8))
                    # rmsnorm: ss = sum over HD of raw^2 (per head)
                    for (raw, nw, kscale, rtag) in (
                            (qraw, qnw_sb, 1.0, 'q'), (kraw, knw_sb, 64.0, 'k')):
                        sq = sqp.tile([128, DIM], BF16, tag='sq')
                        nc.vector.tensor_mul(sq[:], raw[:], raw[:])
                        ss = stp.tile([128, H], F32, tag=f'ss{rtag}')
                        nc.vector.tensor_reduce(
                            ss[:], sq[:].rearrange('p (h e) -> p h e', h=H),
                            mybir.AxisListType.X, mybir.AluOpType.add)
                        nc.vector.tensor_scalar_add(ss[:], ss[:], HD * EPS)
                        nc.vector.reciprocal(ss[:], ss[:])
                        rs = stp.tile([128, H], F32, tag=f'rs{rtag}')
                        nc.scalar.activation(rs[:], ss[:], AF.Sqrt, scale=kscale)
                        # normed = raw * rs (free-bcast) * norm_w -> bf16
                        nn = nrmp.tile([128, DIM], BF16, tag=f'nn{rtag}')
                        rsv = rs[:].unsqueeze(2).broadcast_to([128, H, HD])
                        nc.vector.tensor_mul(
                            nn[:].rearrange('p (h e) -> p h e', h=H),
                            raw[:].rearrange('p (h e) -> p h e', h=H), rsv)
                        nc.vector.tensor_mul(nn[:], nn[:], nw[:])
                        # rope
                        n3 = nn[:].rearrange('p (h e) -> p h e', h=H)
                        ro = ropp.tile([128, DIM], BF16, tag=f'ro{rtag}')
                        r3 = ro[:].rearrange('p (h e) -> p h e', h=H)
                        t2 = ropp.tile([128, DIM], BF16, tag=f't2{rtag}')
                        t3 = t2[:].rearrange('p (h e) -> p h e', h=H)
                        cosv = cos_sb[:, it, :].rearrange('p (h e) -> p h e', h=8)
                        sinv = sin_sb[:, it, :].rearrange('p (h e) -> p h e', h=8)
                        for hh in (slice(0, 8), slice(8, 16)):
                            nc.vector.tensor_mul(r3[:, hh, :], n3[:, hh, :], cosv)
                            nc.vector.tensor_mul(t3[:, hh, 0:RD],
                                                 n3[:, hh, RD:HD],
                                                 sinv[:, :, 0:RD])
                            nc.vector.tensor_mul(t3[:, hh, RD:HD],
                                                 n3[:, hh, 0:RD],
                                                 sinv[:, :, RD:HD])
                        nc.vector.tensor_add(ro[:], ro[:], t2[:])
                        # transpose to [feat, tok] and relayout
                        tt = ttp.tile([128, 8, 128], BF16, tag=f'tt{rtag}')
                        nc.sync.dma_start_transpose(tt[:], ro[:])
                        dstT = qT_all if rtag == 'q' else kT_all
                        nc.vector.tensor_copy(dstT[:, :, it, :], tt[:])

            # ---- P2: attention for this batch (flat SW pipeline: QK/exp of
            # group n is emitted before PV of group n-1 so the PE never
            # waits on the ACT exp of the group it is about to consume) ----
            with tc.tile_pool(name='sps', bufs=2, space='PSUM') as spsp, \
                 tc.tile_pool(name='pvp', bufs=2, space='PSUM') as pvp, \
                 tc.tile_pool(name='ptp', bufs=3) as ptp, \
                 tc.tile_pool(name='rzp', bufs=2) as rzp, \
                 tc.tile_pool(name='aop', bufs=2) as aop:

                def flush(pend):
                    pv, pt, jg, h, ic = pend
                    for i, j in enumerate(jg):
                        nc.tensor.matmul(pv[0:HD + 1, :], v_all[:, j, h, :],
                                         pt[:, bass.ts(i, 512)],
                                         start=(j == 0), stop=(j == 7))
                    if jg[-1] == 7:
                        pb, off = h >> 1, (h & 1) * 64
                        den_b = rzp.tile([1, 512], BF16, tag='den')
                        nc.vector.tensor_copy(den_b[:], pv[HD:HD + 1, :])
                        # broadcast den over 64 rows into pv's spare bank half
                        nc.tensor.matmul(pv[64:128, :], ones_b[:], den_b[:],
                                         start=True, stop=True)
                        rcp = rzp.tile([HD, 512], F32, tag='rcp')
                        nc.vector.reciprocal(rcp[:], pv[64:128, :])
                        ao = aop.tile([HD, 512], BF16, tag='ao')
                        nc.vector.tensor_mul(ao[:], pv[0:HD, :], rcp[:])
                        dst = attnT_d[b, pb, ic * 4:(ic + 1) * 4,
                                      off:off + 64, :].transpose([1, 0, 2])
                        nc.gpsimd.dma_start(
                            dst, ao[:].rearrange('p (i t) -> p i t', i=4))

                pend = None
                for h in range(H):
                    pb, off = h >> 1, (h & 1) * 64
                    kT_h = kT_all[off:off + 64, pb, :, :]   # [64, 8, 128]
                    qT_h = qT_all[off:off + 64, pb, :, :]
                    for ic in range(2):
                        rhs = qT_h[:, ic * 4:(ic + 1) * 4, :]  # [64, 4, 128]
                        pv = pvp.tile([128, 512], F32, tag='pv')
                        for jg in ((0, 1, 2), (3, 4, 5), (6, 7)):
                            w = len(jg) * 512
                            sps = spsp.tile([128, 1536], F32, tag='sps')
                            for i, j in enumerate(jg):
                                nc.tensor.matmul(
                                    sps[:, bass.ts(i, 512)], kT_h[:, j, :],
                                    rhs, start=True, stop=True)
                            pt = ptp.tile([128, 1536], BF16, tag='pt')
                            nc.scalar.activation(pt[:, 0:w], sps[:, 0:w], AF.Exp)
                            if pend is not None:
                                flush(pend)
                            pend = (pv, pt, jg, h, ic)
                flush(pend)

            # ---- P3: out projection for this batch ----
            with tc.tile_pool(name='atp', bufs=3) as atp, \
                 tc.tile_pool(name='yps', bufs=2, space='PSUM') as yps, \
                 tc.tile_pool(name='ysb', bufs=2) as ysb:
                for it in range(TPB):
                    a_t = []
                    for d in range(8):
                        at = atp.tile([128, 128], BF16, tag=f'at{d}')
                        nc.sync.dma_start(at[:], attnT_d[b, d, it, :, :])
                        a_t.append(at)
                    y = ysb.tile([128, DIM], F32, tag='y')
                    for nb in range(2):
                        ps = yps.tile([128, 512], F32, tag='yp')
                        for d in range(8):
                            nc.tensor.matmul(ps[:], a_t[d][:],
                                             wo_sb[d][:, bass.ts(nb, 512)],
                                             start=(d == 0), stop=(d == 7))
                        nc.vector.tensor_add(y[:, bass.ts(nb, 512)], ps[:],
                                             bo_sb[:, bass.ts(nb, 512)])
                    nc.gpsimd.dma_start(
                        out_ap[b * S + it * 128:b * S + (it + 1) * 128, :], y[:])
    nc.compile()
    return nc


_NC_CACHE = None
TRACE = False
LAST_RESULT = None


def kernel(x, w_qkv, b_qkv, q_norm_w, k_norm_w, w_out, b_out):
    global _NC_CACHE, LAST_RESULT
    if _NC_CACHE is None:
        _NC_CACHE = build_graph()
    nc = _NC_CACHE
    bf = ml_dtypes.bfloat16
    cos, sin = _rope_tables()                       # [S, HD] f32
    # tables: [128 part, TPB pos-tiles, 8 heads * HD], sinF sign-folded
    cos_t = cos.reshape(TPB, 128, HD).transpose(1, 0, 2)      # [128, TPB, HD]
    sinF = sin.copy()
    sinF[:, 0:RD] = -sinF[:, 0:RD]
    sin_t = sinF.reshape(TPB, 128, HD).transpose(1, 0, 2)
    cos_b = np.broadcast_to(cos_t[:, :, None, :],
                            (128, TPB, 8, HD)).reshape(128, TPB, 8 * HD)
    sin_b = np.broadcast_to(sin_t[:, :, None, :],
                            (128, TPB, 8, HD)).reshape(128, TPB, 8 * HD)
    com = {
        'wq_b': np.ascontiguousarray(w_qkv, dtype=np.float32).astype(bf),
        'wo_b': np.ascontiguousarray(w_out, dtype=np.float32).astype(bf),
        'bq_b': np.broadcast_to(b_qkv[None, :].astype(np.float32),
                                (128, 3 * DIM)).astype(bf),
        'bo_b': np.broadcast_to(b_out[None, :].astype(np.float32),
                                (128, DIM)).astype(bf),
        'cos_b': np.ascontiguousarray(cos_b).astype(bf),
        'sinF_b': np.ascontiguousarray(sin_b).astype(bf),
        'qnw_b': np.broadcast_to(np.tile(q_norm_w.astype(np.float32), H)[None, :],
                                 (128, DIM)).astype(bf),
        'knw_b': np.broadcast_to(np.tile(k_norm_w.astype(np.float32), H)[None, :],
                                 (128, DIM)).astype(bf),
    }
    x = np.ascontiguousarray(x, dtype=np.float32)
    in_maps = []
    for c in range(NCORES):
        m = dict(com)
        m['x'] = np.ascontiguousarray(x[c * BL:(c + 1) * BL].reshape(T, DIM))
        in_maps.append(m)
    res = run_bass_kernel_spmd(nc, in_maps, core_ids=list(range(NCORES)),
                               trace=TRACE)
    LAST_RESULT = res
    outs = [res.results[c]['out'].reshape(BL, S, DIM) for c in range(NCORES)]
    return np.concatenate(outs, axis=0)
